# revision 9
# baseline (speedup 1.0000x reference)
"""Trainium2 Bass kernel for a dense transformer block (RMSNorm + GQA attention
with RoPE + SwiGLU MLP), distributed over 8 NeuronCores.

Sharding: data-parallel over (batch, query-block). Core c handles batch c//4,
queries [512*(c%4), 512*(c%4+1)). Each core computes K/V for all 2048 keys of
its batch (communication-free); causality is applied via per-core mask data so
the SPMD program is identical on every core.

Device tensors live in transposed layout [feature, token] so contractions sit
on the partition axis. Weights are host-packed into lhsT tile layout. Softmax
runs without max-subtraction (scores have sigma~0.8; exp cannot overflow),
letting attention numerators and denominators accumulate directly in PSUM.

The causal mask is a single [128, 2432] "staircase": the mask tile for key
subtile ks is its slice at offset (15-ks)*128, so one small tensor serves all
16 subtiles and the slice offsets are core-independent.
"""

import sys

sys.path.insert(0, "/opt/trn_rl_repo")

import numpy as np

B, S, D = 2, 2048, 2048
H, KVH, HD = 16, 8, 128
FF = 5504
P = 128
DS = D // P          # 16 subtiles of D
FFC = FF // P        # 43 subtiles of FF
QN = 512             # queries per core
KC = S // 512        # 4 key chunks
NKS = S // P         # 16 key subtiles
MEXT = S + 512 - P   # 2432 staircase width
EPS = 1e-5
NCORES = 8
F_GROUPS = ((0, 11), (11, 22), (22, 33), (33, FFC))

_prog = None


def _build():
    from contextlib import ExitStack

    import concourse.bass as bass  # noqa: F401
    import concourse.tile as tile
    from concourse import bacc, mybir
    from concourse.masks import make_identity

    f32 = mybir.dt.float32
    AF = mybir.ActivationFunctionType
    OP = mybir.AluOpType

    nc = bacc.Bacc("TRN2", target_bir_lowering=False, debug=False)

    xT = nc.dram_tensor("xT", [D, S], f32, kind="ExternalInput").ap()
    xTq = nc.dram_tensor("xTq", [D, QN], f32, kind="ExternalInput").ap()
    wq = nc.dram_tensor("wq_pk", [H, P, DS, P], f32, kind="ExternalInput").ap()
    wk = nc.dram_tensor("wk_pk", [KVH, P, DS, P], f32, kind="ExternalInput").ap()
    wv = nc.dram_tensor("wv_pk", [KVH, P, DS, P], f32, kind="ExternalInput").ap()
    wo = nc.dram_tensor("wo_pk", [DS, P, H, P], f32, kind="ExternalInput").ap()
    wg = nc.dram_tensor("wg_pk", [FFC, P, DS, P], f32, kind="ExternalInput").ap()
    wu = nc.dram_tensor("wu_pk", [FFC, P, DS, P], f32, kind="ExternalInput").ap()
    wd = nc.dram_tensor("wd_pk", [DS, P, FFC, P], f32, kind="ExternalInput").ap()
    cosk = nc.dram_tensor("cos_k", [P, S], f32, kind="ExternalInput").ap()
    sink = nc.dram_tensor("sin_k", [P, S], f32, kind="ExternalInput").ap()
    cosq = nc.dram_tensor("cos_q", [P, QN], f32, kind="ExternalInput").ap()
    sinq = nc.dram_tensor("sin_q", [P, QN], f32, kind="ExternalInput").ap()
    mask = nc.dram_tensor("mask_ext", [P, MEXT], f32, kind="ExternalInput").ap()
    out_rows = nc.dram_tensor("out_rows", [QN, D], f32, kind="ExternalOutput").ap()

    k_spill = nc.dram_tensor("k_spill", [KVH, P, S], f32).ap()
    v_spill = nc.dram_tensor("v_spill", [NKS, P, KVH * P], f32).ap()

    xT_r = xT.rearrange("(ds p) t -> p ds t", p=P)
    xTq_r = xTq.rearrange("(ds p) t -> p ds t", p=P)
    v_spill_r = v_spill.rearrange("kb p n -> p kb n")

    with tile.TileContext(nc) as tc, ExitStack() as ctx:
        # Tag-grouped pools; static SBUF budget/partition ~201KB of 208KB.
        const_pool = ctx.enter_context(tc.tile_pool(name="const", bufs=1))   # ~1.2KB
        big_pool = ctx.enter_context(tc.tile_pool(name="big", bufs=2))       # 64KB
        attn_pool = ctx.enter_context(tc.tile_pool(name="attn", bufs=1))     # 32KB
        mask_pool = ctx.enter_context(tc.tile_pool(name="mask", bufs=1))     # 9.5KB
        hid_pool = ctx.enter_context(tc.tile_pool(name="hid", bufs=1))       # 22KB
        w_pool = ctx.enter_context(tc.tile_pool(name="w", bufs=3))           # 24KB
        kh_pool = ctx.enter_context(tc.tile_pool(name="kh", bufs=1))         # 8KB
        vh_pool = ctx.enter_context(tc.tile_pool(name="vh", bufs=1))         # 8KB
        stage_pool = ctx.enter_context(tc.tile_pool(name="stage", bufs=3))   # 6KB
        sq_pool = ctx.enter_context(tc.tile_pool(name="sq", bufs=2))         # 4KB
        small_pool = ctx.enter_context(tc.tile_pool(name="small", bufs=3))   # 6KB
        rope_pool = ctx.enter_context(tc.tile_pool(name="rope", bufs=3))     # 6KB
        ropec_pool = ctx.enter_context(tc.tile_pool(name="ropec", bufs=3))   # 6KB
        ex_pool = ctx.enter_context(tc.tile_pool(name="ex", bufs=2))         # 4KB
        psum = ctx.enter_context(tc.tile_pool(name="ps", bufs=2, space="PSUM"))

        ones_t = const_pool.tile([P, P], f32, tag="ones")
        nc.vector.memset(ones_t, 1.0)
        ident = const_pool.tile([P, P], f32, tag="ident")
        make_identity(nc, ident)
        eps_t = const_pool.tile([P, 1], f32, tag="eps")
        nc.vector.memset(eps_t, EPS)
        mask_t = mask_pool.tile([P, MEXT], f32, tag="mask")
        nc.sync.dma_start(mask_t, mask)

        def rmsnorm(xt, dst, ncols):
            """dst[:, i, :] = normalized xt[:, i, :]; xt/dst may be the same tile."""
            ps_ss = psum.tile([P, ncols], f32, tag="proj")
            for i in range(DS):
                sq = sq_pool.tile([P, ncols], f32, tag="sq")
                nc.vector.tensor_tensor(sq, xt[:, i, :], xt[:, i, :], OP.mult)
                nc.tensor.matmul(
                    ps_ss, lhsT=ones_t, rhs=sq, start=(i == 0), stop=(i == DS - 1)
                )
            sqv = small_pool.tile([P, ncols], f32, tag="small")
            nc.scalar.activation(sqv, ps_ss, AF.Sqrt, bias=eps_t, scale=1.0 / D)
            rstd = small_pool.tile([P, ncols], f32, tag="small")
            nc.vector.reciprocal(rstd, sqv)
            for i in range(DS):
                nc.vector.tensor_tensor(dst[:, i, :], xt[:, i, :], rstd, OP.mult)

        def rope(ps_in, cos_ap, sin_ap, out_ap):
            """out = ps_in * cos + rotate_half(ps_in) * sin  (sin pre-signed)."""
            a = rope_pool.tile([P, QN], f32, tag="rope")
            nc.vector.tensor_tensor(a, ps_in, cos_ap, OP.mult)
            b = rope_pool.tile([P, QN], f32, tag="rope")
            nc.vector.tensor_tensor(b[0:64, :], ps_in[64:128, :], sin_ap[0:64, :], OP.mult)
            nc.vector.tensor_tensor(b[64:128, :], ps_in[0:64, :], sin_ap[64:128, :], OP.mult)
            nc.vector.tensor_tensor(out_ap, a, b, OP.add)

        # ---------- Phase A: K/V projections over all keys (chunks of 512) ----
        for kc in range(KC):
            ksl = slice(kc * 512, (kc + 1) * 512)
            xt = big_pool.tile([P, DS, 512], f32, tag="big")
            nc.sync.dma_start(xt, xT_r[:, :, ksl])
            rmsnorm(xt, xt, 512)
            cosk_t = ropec_pool.tile([P, 512], f32, tag="ropec")
            nc.sync.dma_start(cosk_t, cosk[:, ksl])
            sink_t = ropec_pool.tile([P, 512], f32, tag="ropec")
            nc.sync.dma_start(sink_t, sink[:, ksl])

            # K projection + RoPE + spill, one 128-dim chunk per KV head
            for kvh in range(KVH):
                wkt = w_pool.tile([P, DS, P], f32, tag="w")
                nc.sync.dma_start(wkt, wk[kvh])
                ps_k = psum.tile([P, 512], f32, tag="score")
                for i in range(DS):
                    nc.tensor.matmul(
                        ps_k, lhsT=wkt[:, i, :], rhs=xt[:, i, :],
                        start=(i == 0), stop=(i == DS - 1),
                    )
                kst = stage_pool.tile([P, 512], f32, tag="stage")
                rope(ps_k, cosk_t, sink_t, kst)
                nc.sync.dma_start(k_spill[kvh][:, ksl], kst)

            # V projection (vT chunks), PE-transpose to [key, dim], spill
            for kvh in range(KVH):
                wvt = w_pool.tile([P, DS, P], f32, tag="w")
                nc.sync.dma_start(wvt, wv[kvh])
                ps_vt = psum.tile([P, 512], f32, tag="att")
                for i in range(DS):
                    nc.tensor.matmul(
                        ps_vt, lhsT=wvt[:, i, :], rhs=xt[:, i, :],
                        start=(i == 0), stop=(i == DS - 1),
                    )
                vts = stage_pool.tile([P, 512], f32, tag="stage")
                nc.scalar.copy(vts, ps_vt)
                for t in range(4):
                    ps_tr = psum.tile([P, P], f32, tag="den")
                    nc.tensor.transpose(ps_tr, vts[:, t * P : (t + 1) * P], ident)
                    trs = stage_pool.tile([P, P], f32, tag="stage")
                    nc.vector.tensor_copy(out=trs, in_=ps_tr)
                    kb = kc * 4 + t
                    nc.sync.dma_start(v_spill[kb][:, kvh * P : (kvh + 1) * P], trs)

        # ---------- Phase A': Q projection + RoPE (own 512 queries) ----------
        xtq = big_pool.tile([P, DS, QN], f32, tag="big")
        nc.sync.dma_start(xtq, xTq_r)
        rmsnorm(xtq, xtq, QN)
        cosq_t = ropec_pool.tile([P, QN], f32, tag="ropec")
        nc.sync.dma_start(cosq_t, cosq)
        sinq_t = ropec_pool.tile([P, QN], f32, tag="ropec")
        nc.sync.dma_start(sinq_t, sinq)
        qrotT = big_pool.tile([P, H, QN], f32, tag="big")
        for h in range(H):
            wqt = w_pool.tile([P, DS, P], f32, tag="w")
            nc.sync.dma_start(wqt, wq[h])
            ps_q = psum.tile([P, QN], f32, tag="score")
            for i in range(DS):
                nc.tensor.matmul(
                    ps_q, lhsT=wqt[:, i, :], rhs=xtq[:, i, :],
                    start=(i == 0), stop=(i == DS - 1),
                )
            rope(ps_q, cosq_t, sinq_t, qrotT[:, h, :])

        # ---------- Phase B: attention ---------------------------------------
        attn_outT = attn_pool.tile([P, H, QN], f32, tag="attn_out")
        kh = None
        vh = None
        for h in range(H):
            kvh = h // 2
            if h % 2 == 0:
                kh = kh_pool.tile([P, S], f32, tag="kh")
                nc.sync.dma_start(kh, k_spill[kvh])
                vh = vh_pool.tile([P, NKS, P], f32, tag="vh")
                nc.sync.dma_start(vh, v_spill_r[:, :, kvh * P : (kvh + 1) * P])
            ps_att = psum.tile([P, QN], f32, tag="att")
            # exp tiles accumulate on DVE (PE has no slack; DVE does), with a
            # single ones-matmul per head for the cross-partition denominator.
            den_acc = stage_pool.tile([P, QN], f32, tag="stage")
            for ks in range(NKS):
                ps_s = psum.tile([P, QN], f32, tag="score")
                nc.tensor.matmul(
                    ps_s, lhsT=kh[:, ks * P : (ks + 1) * P], rhs=qrotT[:, h, :],
                    start=True, stop=True,
                )
                ex = ex_pool.tile([P, QN], f32, tag="ex")
                nc.scalar.activation(ex, ps_s, AF.Exp)
                j0 = (NKS - 1 - ks) * P
                nc.vector.tensor_tensor(ex, ex, mask_t[:, j0 : j0 + QN], OP.mult)
                nc.tensor.matmul(
                    ps_att, lhsT=vh[:, ks, :], rhs=ex,
                    start=(ks == 0), stop=(ks == NKS - 1),
                )
                if ks == 0:
                    nc.vector.tensor_copy(out=den_acc, in_=ex)
                else:
                    nc.vector.tensor_tensor(den_acc, den_acc, ex, OP.add)
            ps_den = psum.tile([P, QN], f32, tag="den")
            nc.tensor.matmul(ps_den, lhsT=ones_t, rhs=den_acc, start=True, stop=True)
            rec = small_pool.tile([P, QN], f32, tag="small")
            nc.vector.reciprocal(rec, ps_den)
            nc.vector.tensor_tensor(attn_outT[:, h, :], ps_att, rec, OP.mult)

        # ---------- Phase C: O projection + residual -------------------------
        yT = big_pool.tile([P, DS, QN], f32, tag="big")
        nc.sync.dma_start(yT, xTq_r)
        for mc in range(DS):
            wot = w_pool.tile([P, H, P], f32, tag="w")
            nc.sync.dma_start(wot, wo[mc])
            ps_o = psum.tile([P, QN], f32, tag="proj")
            for hs in range(H):
                nc.tensor.matmul(
                    ps_o, lhsT=wot[:, hs, :], rhs=attn_outT[:, hs, :],
                    start=(hs == 0), stop=(hs == H - 1),
                )
            nc.vector.tensor_tensor(yT[:, mc, :], yT[:, mc, :], ps_o, OP.add)

        # ---------- Phase D: RMSNorm2 + SwiGLU MLP ---------------------------
        h2T = big_pool.tile([P, DS, QN], f32, tag="big")
        rmsnorm(yT, h2T, QN)

        for f0, f1 in F_GROUPS:
            nf = f1 - f0
            hid = hid_pool.tile([P, 11, QN], f32, tag="hid")
            for j in range(nf):
                ffc = f0 + j
                wgt = w_pool.tile([P, DS, P], f32, tag="w")
                nc.sync.dma_start(wgt, wg[ffc])
                ps_g = psum.tile([P, QN], f32, tag="proj")
                for i in range(DS):
                    nc.tensor.matmul(
                        ps_g, lhsT=wgt[:, i, :], rhs=h2T[:, i, :],
                        start=(i == 0), stop=(i == DS - 1),
                    )
                sg = sq_pool.tile([P, QN], f32, tag="sq")
                nc.scalar.activation(sg, ps_g, AF.Silu)
                wut = w_pool.tile([P, DS, P], f32, tag="w")
                nc.sync.dma_start(wut, wu[ffc])
                ps_u = psum.tile([P, QN], f32, tag="proj")
                for i in range(DS):
                    nc.tensor.matmul(
                        ps_u, lhsT=wut[:, i, :], rhs=h2T[:, i, :],
                        start=(i == 0), stop=(i == DS - 1),
                    )
                nc.vector.tensor_tensor(hid[:, j, :], ps_u, sg, OP.mult)
            for mc in range(DS):
                wdt = w_pool.tile([P, 11, P], f32, tag="w")
                nc.sync.dma_start(wdt[:, :nf, :], wd[mc][:, f0:f1, :])
                ps_d = psum.tile([P, QN], f32, tag="score")
                for j in range(nf):
                    nc.tensor.matmul(
                        ps_d, lhsT=wdt[:, j, :], rhs=hid[:, j, :],
                        start=(j == 0), stop=(j == nf - 1),
                    )
                nc.vector.tensor_tensor(yT[:, mc, :], yT[:, mc, :], ps_d, OP.add)

        # ---------- Phase E: transpose to row layout + store ------------------
        for qs in range(QN // P):
            rows = w_pool.tile([P, DS, P], f32, tag="w")
            for mc in range(DS):
                ps_tr = psum.tile([P, P], f32, tag="den")
                nc.tensor.transpose(ps_tr, yT[:, mc, qs * P : (qs + 1) * P], ident)
                nc.vector.tensor_copy(out=rows[:, mc, :], in_=ps_tr)
            nc.sync.dma_start(out_rows[qs * P : (qs + 1) * P, :], rows)

    nc.compile()
    return nc


def _pack_lhsT(w):
    """[M, K] row-major -> lhsT tile layout:
    out[mc, p, ks, c] = w[mc*128 + c, ks*128 + p]."""
    M, K = w.shape
    w4 = w.reshape(M // P, P, K // P, P)  # [mc, c, ks, p]
    return np.ascontiguousarray(w4.transpose(0, 3, 2, 1))


def _prep_inputs(inputs):
    x = np.asarray(inputs["x"], np.float32)
    cos = np.asarray(inputs["cos"], np.float32)
    sin = np.asarray(inputs["sin"], np.float32)
    g1 = np.asarray(inputs["g1"], np.float32)
    g2 = np.asarray(inputs["g2"], np.float32)

    wq = np.asarray(inputs["wq"], np.float32) * g1[None, :]
    wk = np.asarray(inputs["wk"], np.float32) * g1[None, :]
    wv = np.asarray(inputs["wv"], np.float32) * g1[None, :]
    wo = np.asarray(inputs["wo"], np.float32)
    wgate = np.asarray(inputs["w_gate"], np.float32) * g2[None, :]
    wup = np.asarray(inputs["w_up"], np.float32) * g2[None, :]
    wdown = np.asarray(inputs["w_down"], np.float32)

    shared = {
        "wq_pk": _pack_lhsT(wq),
        "wk_pk": _pack_lhsT(wk),
        "wv_pk": _pack_lhsT(wv),
        "wo_pk": _pack_lhsT(wo),
        "wg_pk": _pack_lhsT(wgate),
        "wu_pk": _pack_lhsT(wup),
        "wd_pk": _pack_lhsT(wdown),
    }
    cosT = np.ascontiguousarray(cos.T)                      # [128, S]
    sinT = sin.T.copy()
    sinT[0:64, :] *= -1.0                                   # pre-signed rotate_half
    sinT = np.ascontiguousarray(sinT)
    shared["cos_k"] = cosT
    shared["sin_k"] = sinT

    xT_b = [np.ascontiguousarray(x[b].T) for b in range(B)]  # [D, S]
    scale = 1.0 / np.sqrt(np.float32(HD))

    in_maps = []
    for c in range(NCORES):
        b, qi = c // 4, c % 4
        q0 = qi * QN
        j = np.arange(MEXT)
        m_ext = (np.arange(P)[:, None] <= (q0 + j - (S - P))[None, :]).astype(np.float32)
        in_maps.append(
            dict(
                shared,
                xT=xT_b[b],
                xTq=np.ascontiguousarray(xT_b[b][:, q0 : q0 + QN]),
                cos_q=np.ascontiguousarray(cosT[:, q0 : q0 + QN] * scale),
                sin_q=np.ascontiguousarray(sinT[:, q0 : q0 + QN] * scale),
                mask_ext=np.ascontiguousarray(m_ext),
            )
        )
    return in_maps


def kernel(**inputs):
    global _prog
    from concourse.bass_utils import run_bass_kernel_spmd

    if _prog is None:
        _prog = _build()
    in_maps = _prep_inputs(inputs)
    res = run_bass_kernel_spmd(_prog, in_maps, list(range(NCORES)))
    out = np.empty((B, S, D), np.float32)
    for c in range(NCORES):
        out[c // 4, (c % 4) * QN : (c % 4 + 1) * QN, :] = res.results[c]["out_rows"]
    return out
